# revision 13
# baseline (speedup 1.0000x reference)
"""Trainium2 Bass kernel for fused Luong 'general' attention.

Reference: energy = enc @ W^T + b; attn_energies[b,s] = hidden[0,b,:].energy;
out = softmax over s -> [B,1,S].

Algebra: with v = hidden[0] @ W, out[b,0,s] = softmax_s(v[b,:].enc[s,b,:]);
the b_attn term is constant in s and cancels under softmax. Data-parallel
over batch B=32 across 8 cores (4 each); host relays out enc to the exact
SBUF layout and casts to fp16 (error budget 2e-2, fp16 measures ~1e-2).

v2 structure (from trace analysis of the 76.5us baseline):
- vT is computed DIRECTLY on PE (stationary = W k-chunk [128,128], moving =
  hidden chunk [128,4] -> out has h on partitions), replacing the baseline's
  v-compute + 8 PE transposes + DVE copies that delayed the first enc matmul
  to +24us.
- The 4 local batches' dot-product streams run CONCURRENTLY in the PE array
  via col-tiling: tile_position=(0,32*bl) puts batch bl's M=1 matmul in its
  own 32-col group; 4 streams share the array (measured 2.4-3x for 3-4 col
  tiles). PE time per 4MB strip drops ~4x vs sequential, making the kernel
  purely DMA-bound.
- Energies land on PSUM partitions {0,32,64,96}: ONE Exp per 512-strip
  covers all 4 batches (was 16 single-lane Exps), with accum_out producing
  the strip sums for free on ACT.
- Rings: W first on both rings, then ALL enc tiles on the sync ring (SP has
  no compute; its queue can block on ring-full harmlessly); hidc+W1 on the
  scalar ring ahead of the Exps. Output stores go on the sync ring, which is
  idle by then (baseline's scalar-ring stores landed 5us after compute).
- Strip 3 is fetched in two half-tiles per batch so the final strip's
  matmuls start before its last bytes land (shorter tail).

Fixed costs (trace): ~6.7us NEFF init (event-sem init + engine table loads)
before the program, ~2.5us event-sem teardown after. DMA floor: 18.06 MB
(16 enc + 2 W + misc) at <=358 GB/s HBM-per-NC = ~50.4us.
"""
import sys
for _p in (
    "/root/.axon_site",
    "/root/.axon_site/_ro/trn_rl_repo",
    "/root/.axon_site/_ro/pypackages",
):
    if _p not in sys.path:
        sys.path.append(_p)

import numpy as np
import concourse.bass as bass
import concourse.tile as tile
from concourse import bacc, mybir
from concourse.bass_utils import run_bass_kernel_spmd

S, B, H = 2048, 32, 1024
N_CORES = 8
B_LOC = B // N_CORES
F32 = mybir.dt.float32
F16 = mybir.dt.float16
P = 128
SCHUNK = 512
NEG_C = -110.0


def build_program(b_loc=B_LOC, h=H, s=S, n_devices=N_CORES):
    assert h % P == 0 and s % SCHUNK == 0
    hc_n = h // P          # 8 h-chunks (contraction partitions)
    sc_n = s // SCHUNK     # 4 strips of 512
    ks = hc_n              # 8 k-chunks for the v matmul
    nc = bacc.Bacc(
        "TRN2", target_bir_lowering=False, debug=False, num_devices=n_devices
    )
    e16 = nc.dram_tensor(
        "e16", [sc_n, P, b_loc, hc_n, SCHUNK], F16, kind="ExternalInput"
    ).ap()
    hidc = nc.dram_tensor(
        "hidc", [P, ks, b_loc], F16, kind="ExternalInput"
    ).ap()
    wrows = nc.dram_tensor("wrows", [P, ks, h], F16, kind="ExternalInput").ap()
    out = nc.dram_tensor("out", [b_loc, s], F32, kind="ExternalOutput").ap()
    with tile.TileContext(nc) as tc:
        with (
            tc.tile_pool(name="consts", bufs=1) as consts,
            tc.tile_pool(name="wp", bufs=2) as wp,
            tc.tile_pool(name="encp", bufs=3) as encp,
            tc.tile_pool(name="ench", bufs=2) as ench,
            tc.tile_pool(name="vps", bufs=1, space="PSUM") as vpsp,
            tc.tile_pool(name="psum", bufs=2, space="PSUM") as psp,
        ):
            negc = consts.tile([P, 1], F32)
            nc.vector.memset(negc, NEG_C)
            # hidden chunks + W first on both rings: W gates the vT preamble
            hidc_sb = consts.tile([P, ks, b_loc], F16)
            nc.scalar.dma_start(out=hidc_sb, in_=hidc)
            hc_qw = ks // 2
            w_tiles = []
            for wi in range(2):
                wt = wp.tile([P, hc_qw, h], F16, tag="w", name=f"w{wi}")
                eng = nc.sync if wi == 0 else nc.scalar
                eng.dma_start(
                    out=wt, in_=wrows[:, wi * hc_qw : (wi + 1) * hc_qw, :]
                )
                w_tiles.append(wt)
            # enc on the sync ring as ONE 4MB DMA per strip (32KB contiguous
            # per-partition runs); strip 3 in two 2MB halves for a short tail
            et = []
            for sc in range(sc_n - 1):
                t = encp.tile(
                    [P, b_loc, hc_n, SCHUNK], F16, tag="e", name=f"et{sc}"
                )
                nc.sync.dma_start(out=t, in_=e16[sc])
                et.append(t)
            hh = hc_n // 2
            et3 = []
            for half in range(2):
                t = ench.tile(
                    [P, b_loc, hh, SCHUNK], F16, tag="eh", name=f"et3{half}"
                )
                nc.sync.dma_start(
                    out=t,
                    in_=e16[sc_n - 1, :, :, half * hh : (half + 1) * hh, :],
                )
                et3.append(t)

            # vT[h_p, hcc, b] = sum_k W[k, hcc*128+h_p] * hidden[b, k]
            # stationary = W k-chunk block (128x128), moving = hidden [128,4]
            # NOTE: start=True clears has_written for the WHOLE PSUM bank, so
            # with accumulation groups interleaved in one bank only the very
            # first matmul may use start=True; the other groups' first writes
            # rely on cleared-bit = overwrite semantics.
            vt_ps = vpsp.tile([P, hc_n * b_loc], F32, tag="v")
            for kl in range(ks):
                wt = w_tiles[kl // hc_qw]
                for hcc in range(hc_n):
                    nc.tensor.matmul(
                        vt_ps[:, hcc * b_loc : (hcc + 1) * b_loc],
                        wt[:, kl % hc_qw, hcc * P : (hcc + 1) * P],
                        hidc_sb[:, kl, :],
                        start=(kl == 0 and hcc == 0),
                        stop=(kl == ks - 1),
                        skip_group_check=True,
                    )
            vh = consts.tile([P, hc_n * b_loc], F16)
            nc.vector.tensor_copy(vh, vt_ps)

            psb = consts.tile([P, s], F32)
            s4 = consts.tile([P, sc_n], F32)
            for sc in range(sc_n):
                eps = psp.tile([P, SCHUNK], F32, tag="ps")
                if sc < sc_n - 1:
                    src = et[sc]

                    def rhs_of(hcc, bl):
                        return src[:, bl, hcc, :]
                else:

                    def rhs_of(hcc, bl):
                        return et3[hcc // hh][:, bl, hcc % hh, :]

                for hcc in range(hc_n):
                    for bl in range(b_loc):
                        nc.tensor.matmul(
                            eps[32 * bl : 32 * bl + 1, :],
                            vh[:, hcc * b_loc + bl : hcc * b_loc + bl + 1],
                            rhs_of(hcc, bl),
                            start=(hcc == 0),
                            stop=(hcc == hc_n - 1),
                            tile_position=(0, 32 * bl),
                            skip_group_check=True,
                        )
                nc.scalar.activation(
                    psb[:, sc * SCHUNK : (sc + 1) * SCHUNK], eps,
                    mybir.ActivationFunctionType.Exp,
                    bias=negc, scale=1.0,
                    accum_out=s4[:, sc : sc + 1],
                )
                if sc == sc_n - 2:
                    # pre-reduce strips 0..2 while strip 3 computes
                    sp = consts.tile([P, 1], F32)
                    nc.vector.tensor_reduce(
                        sp, s4[:, 0 : sc_n - 1],
                        axis=mybir.AxisListType.X, op=mybir.AluOpType.add,
                    )
            ssum = consts.tile([P, 1], F32)
            nc.vector.tensor_tensor(
                ssum, sp, s4[:, sc_n - 1 : sc_n], op=mybir.AluOpType.add
            )
            rinv = consts.tile([P, 1], F32)
            nc.vector.reciprocal(rinv, ssum)
            # normalize split DVE/ACT by their elem rates; each engine's half
            # is stored as ONE partition-strided DMA issued from its own ring
            # queue so the two doorbells run in parallel
            cut = 1280
            nc.vector.tensor_scalar_mul(psb[:, 0:cut], psb[:, 0:cut], rinv)
            nc.scalar.mul(psb[:, cut:s], psb[:, cut:s], rinv)
            nc.sync.dma_start(
                out=out[:, 0:cut], in_=psb[0 : 32 * b_loc : 32, 0:cut]
            )
            nc.scalar.dma_start(
                out=out[:, cut:s], in_=psb[0 : 32 * b_loc : 32, cut:s]
            )
    nc.compile()
    return nc


def _make_in_maps(hidden, encoder_outputs, W_attn):
    hidden = np.ascontiguousarray(np.asarray(hidden, dtype=np.float32))
    enc = np.asarray(encoder_outputs, dtype=np.float32)
    W = np.ascontiguousarray(np.asarray(W_attn, dtype=np.float32))
    hc_n = H // P
    sc_n = S // SCHUNK
    e16 = np.ascontiguousarray(
        enc.reshape(sc_n, SCHUNK, B, hc_n, P).transpose(0, 4, 2, 3, 1)
    ).astype(np.float16)
    hid_r = hidden[0].T.reshape(hc_n, P, B)
    hid16 = hid_r.transpose(1, 0, 2).astype(np.float16)
    w16 = np.ascontiguousarray(
        W.reshape(hc_n, P, H).transpose(1, 0, 2)
    ).astype(np.float16)
    in_maps = []
    for i in range(N_CORES):
        lo, hi = i * B_LOC, (i + 1) * B_LOC
        in_maps.append(
            {
                "e16": np.ascontiguousarray(e16[:, :, lo:hi]),
                "hidc": np.ascontiguousarray(hid16[:, :, lo:hi]),
                "wrows": w16,
            }
        )
    return in_maps


def run_spmd(hidden, encoder_outputs, W_attn, b_attn=None, trace=False):
    in_maps = _make_in_maps(hidden, encoder_outputs, W_attn)
    nc = build_program()
    res = run_bass_kernel_spmd(nc, in_maps, list(range(N_CORES)), trace=trace)
    out = np.concatenate([r["out"] for r in res.results], axis=0)
    return np.ascontiguousarray(out[:, None, :].astype(np.float32)), res


def kernel(hidden, encoder_outputs, W_attn, b_attn):
    out, _ = run_spmd(hidden, encoder_outputs, W_attn, b_attn)
    return out


# revision 14
# speedup vs baseline: 1.1030x; 1.1030x over previous
"""Trainium2 Bass kernel for fused Luong 'general' attention.

Reference: energy = enc @ W^T + b; attn_energies[b,s] = hidden[0,b,:].energy;
out = softmax over s -> [B,1,S].

Algebra: with v = hidden[0] @ W, out[b,0,s] = softmax_s(v[b,:].enc[s,b,:]);
the b_attn term is constant in s and cancels under softmax. Data-parallel
over batch B=32 across 8 cores (4 each); host relays out enc to the exact
SBUF layout and casts to fp16 (error budget 2e-2, fp16 measures ~1e-2).

Structure (evolved from a 76.5us baseline via trace analysis):
- vT computed DIRECTLY on PE (stationary = W k-chunk [128,128], moving =
  hidden chunk [128,4] -> h lands on partitions), replacing v-compute +
  8 PE transposes + DVE copies that delayed the first enc matmul by ~11us.
- The 4 local batches' M=1 dot-product streams run CONCURRENTLY in the PE
  array via col-tiling (tile_position=(0,32*bl)); PE drops to ~20% busy and
  the kernel is purely DMA/fabric-bound.
- PSUM semantics: start=True clears has_written bits for the bank ON THE
  PARTITIONS THE MATMUL WRITES. Partition-disjoint groups (enc batches) each
  use their own start=True; groups sharing partitions at different offsets
  (vT) may only use start=True on the very first matmul of the bank.
- Energies land on PSUM partitions {0,32,64,96}: one Exp per strip covers
  all 4 batches, accum_out gives strip sums for free on ACT.
- DMA: enc strips 0-2 as ONE 12MB partition-major DMA (48KB contiguous runs
  per partition, ~400 GB/s measured) + the last 512 columns as two 2MB
  256-col sub-strips so the tail exp halves and overlaps the final matmuls.
  All enc on the sync ring (SP has no compute, its queue may block on
  ring-full harmlessly); hidc+W on the scalar ring ahead of the Exps.
- Epilogue: normalize split DVE(1280)/ACT(768) by engine rate, each half
  stored as one partition-strided DMA from its own ring so the doorbells
  (~0.6us each) run in parallel.

Fixed costs (trace): ~6.7us NEFF init before the program + ~1.5us to first
DMA byte, ~2.5us event-sem teardown after the last store. Contended mode
adds a straggler DMA engine (~10% slower + late start) that gates strip
completion sems; total-byte reduction is the only lever against it.
"""
import sys
for _p in (
    "/root/.axon_site",
    "/root/.axon_site/_ro/trn_rl_repo",
    "/root/.axon_site/_ro/pypackages",
):
    if _p not in sys.path:
        sys.path.append(_p)

import numpy as np
import concourse.bass as bass
import concourse.tile as tile
from concourse import bacc, mybir
from concourse.bass_utils import run_bass_kernel_spmd

S, B, H = 2048, 32, 1024
N_CORES = 8
B_LOC = B // N_CORES
F32 = mybir.dt.float32
F16 = mybir.dt.float16
P = 128
SCHUNK = 512
SUB = 256           # width of the two tail sub-strips
NEG_C = -110.0


def build_program(b_loc=B_LOC, h=H, s=S, n_devices=N_CORES):
    assert h % P == 0 and s % SCHUNK == 0
    hc_n = h // P          # 8 h-chunks (contraction partitions)
    sc_n = s // SCHUNK     # 4 x 512 logical strips
    scf = sc_n - 1         # full strips fetched in the big DMA
    ks = hc_n              # 8 k-chunks for the v matmul
    nc = bacc.Bacc(
        "TRN2", target_bir_lowering=False, debug=False, num_devices=n_devices
    )
    e16a = nc.dram_tensor(
        "e16a", [P, scf, b_loc, hc_n, SCHUNK], F16, kind="ExternalInput"
    ).ap()
    e16b = nc.dram_tensor(
        "e16b", [P, 2, b_loc, hc_n, SUB], F16, kind="ExternalInput"
    ).ap()
    hidc = nc.dram_tensor(
        "hidc", [P, ks, b_loc], F16, kind="ExternalInput"
    ).ap()
    wrows = nc.dram_tensor("wrows", [P, ks, h], F16, kind="ExternalInput").ap()
    out = nc.dram_tensor("out", [b_loc, s], F32, kind="ExternalOutput").ap()
    with tile.TileContext(nc) as tc:
        with (
            tc.tile_pool(name="consts", bufs=1) as consts,
            tc.tile_pool(name="wp", bufs=1) as wp,
            tc.tile_pool(name="encp", bufs=1) as encp,
            tc.tile_pool(name="ench", bufs=2) as ench,
            tc.tile_pool(name="vps", bufs=1, space="PSUM") as vpsp,
            tc.tile_pool(name="psum", bufs=2, space="PSUM") as psp,
        ):
            negc = consts.tile([P, 1], F32)
            nc.vector.memset(negc, NEG_C)
            hidc_sb = consts.tile([P, ks, b_loc], F16)
            nc.scalar.dma_start(out=hidc_sb, in_=hidc)
            w_t = wp.tile([P, ks, h], F16, tag="w")
            nc.scalar.dma_start(out=w_t, in_=wrows)
            # enc: one 12MB DMA for strips 0-2, two 2MB tail sub-strips
            et_a = encp.tile([P, scf, b_loc, hc_n, SCHUNK], F16, tag="e")
            nc.sync.dma_start(out=et_a, in_=e16a)
            et3 = []
            for half in range(2):
                t = ench.tile([P, b_loc, hc_n, SUB], F16, tag="eh",
                              name=f"et3{half}")
                nc.sync.dma_start(out=t, in_=e16b[:, half])
                et3.append(t)

            # vT[h_p, hcc, b] = sum_k W[k, hcc*128+h_p] * hidden[b, k]
            vt_ps = vpsp.tile([P, hc_n * b_loc], F32, tag="v")
            for kl in range(ks):
                for hcc in range(hc_n):
                    nc.tensor.matmul(
                        vt_ps[:, hcc * b_loc : (hcc + 1) * b_loc],
                        w_t[:, kl, hcc * P : (hcc + 1) * P],
                        hidc_sb[:, kl, :],
                        start=(kl == 0 and hcc == 0),
                        stop=(kl == ks - 1),
                        skip_group_check=True,
                    )
            vh = consts.tile([P, hc_n * b_loc], F16)
            nc.vector.tensor_copy(vh, vt_ps)

            # strips: (psb column offset, width, rhs accessor)
            strips = [
                (sc * SCHUNK, SCHUNK,
                 (lambda sc: lambda hcc, bl: et_a[:, sc, bl, hcc, :])(sc))
                for sc in range(scf)
            ] + [
                (scf * SCHUNK + j * SUB, SUB,
                 (lambda j: lambda hcc, bl: et3[j][:, bl, hcc, :])(j))
                for j in range(2)
            ]
            n_str = len(strips)
            psb = consts.tile([P, s], F32)
            s4 = consts.tile([P, n_str], F32)
            for si, (off, width, rhs_of) in enumerate(strips):
                epsb = psp.tile([P, SCHUNK], F32, tag="ps")
                eps = epsb[:, 0:width]
                for hcc in range(hc_n):
                    for bl in range(b_loc):
                        nc.tensor.matmul(
                            eps[32 * bl : 32 * bl + 1, :],
                            vh[:, hcc * b_loc + bl : hcc * b_loc + bl + 1],
                            rhs_of(hcc, bl),
                            start=(hcc == 0),
                            stop=(hcc == hc_n - 1),
                            tile_position=(0, 32 * bl),
                            skip_group_check=True,
                        )
                nc.scalar.activation(
                    psb[:, off : off + width], eps,
                    mybir.ActivationFunctionType.Exp,
                    bias=negc, scale=1.0,
                    accum_out=s4[:, si : si + 1],
                )
                if si == n_str - 2:
                    # pre-reduce all but the last sub-strip's sums
                    sp = consts.tile([P, 1], F32)
                    nc.vector.tensor_reduce(
                        sp, s4[:, 0 : n_str - 1],
                        axis=mybir.AxisListType.X, op=mybir.AluOpType.add,
                    )
            ssum = consts.tile([P, 1], F32)
            nc.vector.tensor_tensor(
                ssum, sp, s4[:, n_str - 1 : n_str], op=mybir.AluOpType.add
            )
            rinv = consts.tile([P, 1], F32)
            nc.vector.reciprocal(rinv, ssum)
            # normalize split DVE/ACT by engine elem rate; each half stored as
            # ONE partition-strided DMA from its own ring queue so the two
            # doorbells run in parallel
            cut = 1280
            nc.vector.tensor_scalar_mul(psb[:, 0:cut], psb[:, 0:cut], rinv)
            nc.scalar.mul(psb[:, cut:s], psb[:, cut:s], rinv)
            nc.sync.dma_start(
                out=out[:, 0:cut], in_=psb[0 : 32 * b_loc : 32, 0:cut]
            )
            nc.scalar.dma_start(
                out=out[:, cut:s], in_=psb[0 : 32 * b_loc : 32, cut:s]
            )
    nc.compile()
    return nc


def _make_in_maps(hidden, encoder_outputs, W_attn):
    hidden = np.ascontiguousarray(np.asarray(hidden, dtype=np.float32))
    enc = np.asarray(encoder_outputs, dtype=np.float32)
    W = np.ascontiguousarray(np.asarray(W_attn, dtype=np.float32))
    hc_n = H // P
    sc_n = S // SCHUNK
    scf = sc_n - 1
    # enc5 dims: [sc, s', b, hc, p]
    enc5 = enc.reshape(sc_n, SCHUNK, B, hc_n, P)
    e16a = np.ascontiguousarray(
        enc5[:scf].transpose(4, 0, 2, 3, 1)
    ).astype(np.float16)                      # [P, scf, B, hc, 512]
    e16b = np.ascontiguousarray(
        enc5[scf].reshape(2, SUB, B, hc_n, P).transpose(4, 0, 2, 3, 1)
    ).astype(np.float16)                      # [P, 2, B, hc, 256]
    hid_r = hidden[0].T.reshape(hc_n, P, B)
    hid16 = hid_r.transpose(1, 0, 2).astype(np.float16)
    w16 = np.ascontiguousarray(
        W.reshape(hc_n, P, H).transpose(1, 0, 2)
    ).astype(np.float16)
    in_maps = []
    for i in range(N_CORES):
        lo, hi = i * B_LOC, (i + 1) * B_LOC
        in_maps.append(
            {
                "e16a": np.ascontiguousarray(e16a[:, :, lo:hi]),
                "e16b": np.ascontiguousarray(e16b[:, :, lo:hi]),
                "hidc": np.ascontiguousarray(hid16[:, :, lo:hi]),
                "wrows": w16,
            }
        )
    return in_maps


def run_spmd(hidden, encoder_outputs, W_attn, b_attn=None, trace=False):
    in_maps = _make_in_maps(hidden, encoder_outputs, W_attn)
    nc = build_program()
    res = run_bass_kernel_spmd(nc, in_maps, list(range(N_CORES)), trace=trace)
    out = np.concatenate([r["out"] for r in res.results], axis=0)
    return np.ascontiguousarray(out[:, None, :].astype(np.float32)), res


def kernel(hidden, encoder_outputs, W_attn, b_attn):
    out, _ = run_spmd(hidden, encoder_outputs, W_attn, b_attn)
    return out


# revision 15
# speedup vs baseline: 1.1319x; 1.0262x over previous
"""Trainium2 Bass kernel for fused Luong 'general' attention.

Reference: energy = enc @ W^T + b; attn_energies[b,s] = hidden[0,b,:].energy;
out = softmax over s -> [B,1,S].

Algebra: with v = hidden[0] @ W, out[b,0,s] = softmax_s(v[b,:].enc[s,b,:]);
the b_attn term is constant in s and cancels under softmax. Data-parallel
over batch B=32 across 8 cores (4 each); host relays out enc to the exact
SBUF layout and casts to fp16 (error budget 2e-2, fp16 measures ~1e-2).

Structure (evolved from a 76.5us baseline via trace analysis):
- vT computed DIRECTLY on PE (stationary = W k-chunk [128,128], moving =
  hidden chunk [128,4] -> h lands on partitions), replacing v-compute +
  8 PE transposes + DVE copies that delayed the first enc matmul by ~11us.
- The 4 local batches' M=1 dot-product streams run CONCURRENTLY in the PE
  array via col-tiling (tile_position=(0,32*bl)); PE drops to ~20% busy and
  the kernel is purely DMA/fabric-bound.
- PSUM semantics: start=True clears has_written bits for the bank ON THE
  PARTITIONS THE MATMUL WRITES. Partition-disjoint groups (enc batches) each
  use their own start=True; groups sharing partitions at different offsets
  (vT) may only use start=True on the very first matmul of the bank.
- Energies land on PSUM partitions {0,32,64,96}: one Exp per strip covers
  all 4 batches, accum_out gives strip sums for free on ACT.
- DMA: enc strips 0-2 as ONE 12MB partition-major DMA (48KB contiguous runs
  per partition, ~400 GB/s measured) + the last 512 columns as two 2MB
  256-col sub-strips so the tail exp halves and overlaps the final matmuls.
  All enc on the sync ring (SP has no compute, its queue may block on
  ring-full harmlessly); hidc+W on the scalar ring ahead of the Exps.
- Epilogue: normalize split DVE(1280)/ACT(768) by engine rate, each half
  stored as one partition-strided DMA from its own ring so the doorbells
  (~0.6us each) run in parallel.

Fixed costs (trace): ~6.7us NEFF init before the program + ~1.5us to first
DMA byte, ~2.5us event-sem teardown after the last store. Contended mode
adds a straggler DMA engine (~10% slower + late start) that gates strip
completion sems; total-byte reduction is the only lever against it.
"""
import sys
for _p in (
    "/root/.axon_site",
    "/root/.axon_site/_ro/trn_rl_repo",
    "/root/.axon_site/_ro/pypackages",
):
    if _p not in sys.path:
        sys.path.append(_p)

import numpy as np
import concourse.bass as bass
import concourse.tile as tile
from concourse import bacc, mybir
from concourse.bass_utils import run_bass_kernel_spmd

S, B, H = 2048, 32, 1024
N_CORES = 8
B_LOC = B // N_CORES
F32 = mybir.dt.float32
F16 = mybir.dt.float16
P = 128
SCHUNK = 512
SUB = 256           # width of the two tail sub-strips
NEG_C = -110.0


def build_program(b_loc=B_LOC, h=H, s=S, n_devices=N_CORES):
    assert h % P == 0 and s % SCHUNK == 0
    hc_n = h // P          # 8 h-chunks (contraction partitions)
    sc_n = s // SCHUNK     # 4 x 512 logical strips
    scf = sc_n - 1         # full strips fetched in the big DMA
    ks = hc_n              # 8 k-chunks for the v matmul
    nc = bacc.Bacc(
        "TRN2", target_bir_lowering=False, debug=False, num_devices=n_devices
    )
    e16a = nc.dram_tensor(
        "e16a", [P, scf, b_loc, hc_n, SCHUNK], F16, kind="ExternalInput"
    ).ap()
    e16b = nc.dram_tensor(
        "e16b", [P, 2, b_loc, hc_n, SUB], F16, kind="ExternalInput"
    ).ap()
    hidc = nc.dram_tensor(
        "hidc", [P, ks, b_loc], F16, kind="ExternalInput"
    ).ap()
    wrows = nc.dram_tensor("wrows", [P, ks, h], F16, kind="ExternalInput").ap()
    out = nc.dram_tensor("out", [b_loc, s], F32, kind="ExternalOutput").ap()
    with tile.TileContext(nc) as tc:
        with (
            tc.tile_pool(name="consts", bufs=1) as consts,
            tc.tile_pool(name="wp", bufs=1) as wp,
            tc.tile_pool(name="encp", bufs=1) as encp,
            tc.tile_pool(name="ench", bufs=2) as ench,
            tc.tile_pool(name="vps", bufs=1, space="PSUM") as vpsp,
            tc.tile_pool(name="psum", bufs=2, space="PSUM") as psp,
        ):
            negc = consts.tile([P, 1], F32)
            nc.vector.memset(negc, NEG_C)
            hidc_sb = consts.tile([P, ks, b_loc], F16)
            nc.scalar.dma_start(out=hidc_sb, in_=hidc)
            # W FIRST on the sync ring: ring FIFO guarantees its 2MB drains
            # before the enc stream (on the scalar ring it starves under
            # packet-RR against the sync ring's big descriptors)
            w_t = wp.tile([P, ks, h], F16, tag="w")
            nc.sync.dma_start(out=w_t, in_=wrows)
            # enc: one 4MB DMA per strip (progressive completion), then the
            # two 2MB tail sub-strips
            et_a = encp.tile([P, scf, b_loc, hc_n, SCHUNK], F16, tag="e")
            for sc in range(scf):
                nc.sync.dma_start(out=et_a[:, sc], in_=e16a[:, sc])
            et3 = []
            for half in range(2):
                t = ench.tile([P, b_loc, hc_n, SUB], F16, tag="eh",
                              name=f"et3{half}")
                nc.sync.dma_start(out=t, in_=e16b[:, half])
                et3.append(t)

            # vT[h_p, hcc, b] = sum_k W[k, hcc*128+h_p] * hidden[b, k]
            vt_ps = vpsp.tile([P, hc_n * b_loc], F32, tag="v")
            for kl in range(ks):
                for hcc in range(hc_n):
                    nc.tensor.matmul(
                        vt_ps[:, hcc * b_loc : (hcc + 1) * b_loc],
                        w_t[:, kl, hcc * P : (hcc + 1) * P],
                        hidc_sb[:, kl, :],
                        start=(kl == 0 and hcc == 0),
                        stop=(kl == ks - 1),
                        skip_group_check=True,
                    )
            vh = consts.tile([P, hc_n * b_loc], F16)
            nc.vector.tensor_copy(vh, vt_ps)

            # strips: (psb column offset, width, rhs accessor)
            strips = [
                (sc * SCHUNK, SCHUNK,
                 (lambda sc: lambda hcc, bl: et_a[:, sc, bl, hcc, :])(sc))
                for sc in range(scf)
            ] + [
                (scf * SCHUNK + j * SUB, SUB,
                 (lambda j: lambda hcc, bl: et3[j][:, bl, hcc, :])(j))
                for j in range(2)
            ]
            n_str = len(strips)
            psb = consts.tile([P, s], F32)
            s4 = consts.tile([P, n_str], F32)
            for si, (off, width, rhs_of) in enumerate(strips):
                epsb = psp.tile([P, SCHUNK], F32, tag="ps")
                eps = epsb[:, 0:width]
                for hcc in range(hc_n):
                    for bl in range(b_loc):
                        nc.tensor.matmul(
                            eps[32 * bl : 32 * bl + 1, :],
                            vh[:, hcc * b_loc + bl : hcc * b_loc + bl + 1],
                            rhs_of(hcc, bl),
                            start=(hcc == 0),
                            stop=(hcc == hc_n - 1),
                            tile_position=(0, 32 * bl),
                            skip_group_check=True,
                        )
                nc.scalar.activation(
                    psb[:, off : off + width], eps,
                    mybir.ActivationFunctionType.Exp,
                    bias=negc, scale=1.0,
                    accum_out=s4[:, si : si + 1],
                )
                if si == n_str - 2:
                    # pre-reduce all but the last sub-strip's sums
                    sp = consts.tile([P, 1], F32)
                    nc.vector.tensor_reduce(
                        sp, s4[:, 0 : n_str - 1],
                        axis=mybir.AxisListType.X, op=mybir.AluOpType.add,
                    )
            ssum = consts.tile([P, 1], F32)
            nc.vector.tensor_tensor(
                ssum, sp, s4[:, n_str - 1 : n_str], op=mybir.AluOpType.add
            )
            rinv = consts.tile([P, 1], F32)
            nc.vector.reciprocal(rinv, ssum)
            # normalize split DVE/ACT by engine elem rate; each half stored as
            # ONE partition-strided DMA from its own ring queue so the two
            # doorbells run in parallel
            cut = 1280
            nc.vector.tensor_scalar_mul(psb[:, 0:cut], psb[:, 0:cut], rinv)
            nc.scalar.mul(psb[:, cut:s], psb[:, cut:s], rinv)
            nc.sync.dma_start(
                out=out[:, 0:cut], in_=psb[0 : 32 * b_loc : 32, 0:cut]
            )
            nc.scalar.dma_start(
                out=out[:, cut:s], in_=psb[0 : 32 * b_loc : 32, cut:s]
            )
    nc.compile()
    return nc


def _make_in_maps(hidden, encoder_outputs, W_attn):
    hidden = np.ascontiguousarray(np.asarray(hidden, dtype=np.float32))
    enc = np.asarray(encoder_outputs, dtype=np.float32)
    W = np.ascontiguousarray(np.asarray(W_attn, dtype=np.float32))
    hc_n = H // P
    sc_n = S // SCHUNK
    scf = sc_n - 1
    # enc5 dims: [sc, s', b, hc, p]
    enc5 = enc.reshape(sc_n, SCHUNK, B, hc_n, P)
    e16a = np.ascontiguousarray(
        enc5[:scf].transpose(4, 0, 2, 3, 1)
    ).astype(np.float16)                      # [P, scf, B, hc, 512]
    e16b = np.ascontiguousarray(
        enc5[scf].reshape(2, SUB, B, hc_n, P).transpose(4, 0, 2, 3, 1)
    ).astype(np.float16)                      # [P, 2, B, hc, 256]
    hid_r = hidden[0].T.reshape(hc_n, P, B)
    hid16 = hid_r.transpose(1, 0, 2).astype(np.float16)
    w16 = np.ascontiguousarray(
        W.reshape(hc_n, P, H).transpose(1, 0, 2)
    ).astype(np.float16)
    in_maps = []
    for i in range(N_CORES):
        lo, hi = i * B_LOC, (i + 1) * B_LOC
        in_maps.append(
            {
                "e16a": np.ascontiguousarray(e16a[:, :, lo:hi]),
                "e16b": np.ascontiguousarray(e16b[:, :, lo:hi]),
                "hidc": np.ascontiguousarray(hid16[:, :, lo:hi]),
                "wrows": w16,
            }
        )
    return in_maps


def run_spmd(hidden, encoder_outputs, W_attn, b_attn=None, trace=False):
    in_maps = _make_in_maps(hidden, encoder_outputs, W_attn)
    nc = build_program()
    res = run_bass_kernel_spmd(nc, in_maps, list(range(N_CORES)), trace=trace)
    out = np.concatenate([r["out"] for r in res.results], axis=0)
    return np.ascontiguousarray(out[:, None, :].astype(np.float32)), res


def kernel(hidden, encoder_outputs, W_attn, b_attn):
    out, _ = run_spmd(hidden, encoder_outputs, W_attn, b_attn)
    return out
